# revision 2
# baseline (speedup 1.0000x reference)
"""Echo State Network Bass kernel for Trainium2 (8 NeuronCores, SPMD) — v3.

Problem: x [B=32, C=4, T=512, I=64], input_weights Wi [R=1024, C*I=256],
reservoir_weights W [R=1024, R]. Output [B, C, T, R] f32.

    u_t = flatten(x[:,:,t,:]) @ Wi.T                     (broadcast over C)
    h_t = 0.5*tanh(u_t + h_{t-1} @ W) + 0.5*h_{t-1}      (per (b, c) row)

Sharding: data-parallel over batch, 4 batches/core -> ROWS = 16 independent
reservoir rows per core; time recurrence local.

v3 device algorithm ("pre-activation recurrence", all fp16 in the loop):
  Let a_t = u_t + h_{t-1}@W (the tanh argument), g_t = tanh(a_t),
  s_t = 2*h_t. Then:
      a_{t+1} = u~_{t+1} + g_t@(W/2) + 0.5*a_t,  u~_t = u_t - 0.5*u_{t-1}
      s_t     = 0.5*s_{t-1} + g_t                (output: h = s/2, on host)
  Per step the PSUM accumulation is: identity-matmuls of the staged u~
  slice, W/2-matmuls of g_{t-1} (transposed layout, r on partitions), and
  0.5*identity-matmuls of a16_{t-1} (an fp16 SBUF mirror of the previous
  PSUM, copied off by the *GPSIMD* engine, whose short latency keeps the
  a-feedback off the long Act chain). Act only does the two half-width
  tanhs; DVE only does one fused scalar_tensor_tensor blend per step
  (written straight into the fp16 DMA slab) plus one staging copy per pair.
  The host undoes the slab layout, halves, and converts to fp32.
"""

import os
import sys

import numpy as np

sys.path.insert(0, "/opt/trn_rl_repo")

from contextlib import ExitStack

import concourse.bass as bass
import concourse.tile as tile
from concourse import mybir
from concourse.masks import make_identity

F32 = mybir.dt.float32
F16 = mybir.dt.float16
AF = mybir.ActivationFunctionType
ALU = mybir.AluOpType


def _patched_drain_and_barrier(self, tick_clock, wait_clock):
    # The stock kernel-tail drain carries one sync-wait per touched semaphore;
    # this walrus build caps sync waits per TPB_CTRL instruction, so chunk the
    # waits across several sequential drains on the sync engine.
    from concourse.vector_clock import ScopedClock

    nc = self.nc
    carrier = nc.sync.drain()
    wait_clock.add_sem_waits(
        carrier.ins, ScopedClock({None: tick_clock.global_clock})
    )
    si = carrier.ins.sync_info
    waits = list(si.on_wait) if si is not None else []
    if len(waits) > 1:
        carrier.ins.sync_info.on_wait = waits[:1]
        for w in waits[1:]:
            d2 = nc.sync.drain()
            d2.ins.sync_info = mybir.SyncInfo(on_wait=[w], on_update=[])
    nc.all_engine_barrier()
    popped = nc._tile_sem_poison_stack.pop()
    assert popped is self._sem_poison
    nc.clear_and_free_semaphores(list(self.sems.allocated().values()))
    nc.all_engine_barrier()


tile.TileContext._drain_and_barrier = _patched_drain_and_barrier

_MAX_SYNC_WAITS = 1


def _split_sync_waits(nc):
    """This walrus build rejects instructions carrying more than a couple of
    sync waits. Move excess waits onto same-engine NoOp carriers inserted
    immediately before the instruction (sem thresholds are absolute, so
    waiting earlier in the same engine stream is equivalent)."""
    import copy

    scratch = bass.Bass("TRN2", target_bir_lowering=False, debug=False)
    with scratch.Block() as blk:

        @blk.sync
        def _(sync):
            sync.nop(hint="waitcarrier")

    template = None
    for bb in scratch.m.functions[0].blocks:
        for i in bb.instructions:
            if i.opcode == "NoOp":
                template = i
    assert template is not None

    n_added = 0
    for f in nc.m.functions:
        for bb in f.blocks:
            out = []
            for inst in bb.instructions:
                si = inst.sync_info
                waits = list(si.on_wait) if si is not None else []
                if len(waits) > _MAX_SYNC_WAITS:
                    extra = waits[: -_MAX_SYNC_WAITS]
                    for w in extra:
                        nop = copy.copy(template)
                        n_added += 1
                        nop.name = f"I-wsplit{n_added}"
                        nop.engine = inst.engine
                        nop.sync_info = mybir.SyncInfo(on_wait=[w], on_update=[])
                        out.append(nop)
                    inst.sync_info.on_wait = waits[-_MAX_SYNC_WAITS:]
                out.append(inst)
            if n_added:
                bb.instructions[:] = out
    return n_added


B, C, T, I, R = 32, 4, 512, 64, 1024
NCORES = 8
BL = B // NCORES          # 4 local batches per core
ROWS = BL * C             # 16 rows; row = b*C + c
KC = R // 128             # 8 contraction chunks
MC = R // 128             # 8 output blocks
CI = C * I                # 256
CIC = CI // 128           # 2 ci chunks


def _cbroad(ap4):
    """Append a stride-0 C dim to a [...,BL] AP -> rows (b, c) broadcast."""
    return bass.AP(ap4.tensor, ap4.offset, list(ap4.ap) + [[0, C]])


def _precompute_u(ctx, tc, x, wi, u_sb, tval, after_x_dmas=None):
    """u~ into SBUF fp16: u_sb[p, t, m, b] = u_t - 0.5*u_{t-1} at r=128m+p.

    The -0.5*u_{t-1} term is folded into the PE accumulation (a second set
    of matmuls against -0.5*WiT with the x operand shifted one step), so the
    psum holds u~ directly and one copy per block finishes the job."""
    nc = tc.nc
    with (
        tc.tile_pool(name="pre", bufs=1) as pre,
        tc.tile_pool(name="pps", bufs=4, space="PSUM") as pps,
    ):
        ident = pre.tile([128, 128], F32, tag="ident")
        make_identity(nc, ident)

        # Wi natural [r-block, ci] -> PE-transpose -> WiT fp16 [ci, r] and
        # the -0.5-scaled copy WiTn (used for the shifted u~ term).
        # One merged DMA: per-instruction SP issue cost dominates small DMAs.
        win_all = pre.tile([128, MC, CI], F32, tag="win", name="win_all")
        nc.sync.dma_start(
            out=win_all, in_=wi.rearrange("(m p) c -> p m c", p=128)
        )
        win = [win_all[:, m, :] for m in range(MC)]
        wiT = [
            pre.tile([128, R], F16, tag=f"wit{c}", name=f"wit{c}")
            for c in range(CIC)
        ]
        wiTn = [
            pre.tile([128, R], F16, tag=f"witn{c}", name=f"witn{c}")
            for c in range(CIC)
        ]
        for m in range(MC):
            for ck in range(CIC):
                pt = pps.tile([128, 128], F32, tag="pt")
                nc.tensor.transpose(pt, win[m][:, 128 * ck : 128 * (ck + 1)], ident)
                nc.vector.tensor_copy(wiT[ck][:, 128 * m : 128 * (m + 1)], pt)
                nc.scalar.activation(
                    wiTn[ck][:, 128 * m : 128 * (m + 1)],
                    pt,
                    AF.Copy,
                    scale=-0.5,
                )

        # xT fp16 [(c2 i) part, (t, b) free] per ci chunk; one DMA per
        # (chunk, channel-pair) instead of per batch
        xT32 = [
            pre.tile([128, tval, BL], F32, tag=f"xt{c}", name=f"xt{c}")
            for c in range(CIC)
        ]
        qs = [nc.sync, nc.scalar]
        di = 0
        for ck in range(CIC):
            for cv in range(2):
                for b in range(BL):
                    src = x[b, 2 * ck + cv, :, :].rearrange("t i -> i t")
                    qs[di % 2].dma_start(
                        out=xT32[ck][I * cv : I * (cv + 1), :, b], in_=src
                    )
                    di += 1
        if after_x_dmas is not None:
            after_x_dmas()
        xT = [
            pre.tile([128, tval * BL], F16, tag=f"xh{c}", name=f"xh{c}")
            for c in range(CIC)
        ]
        nc.vector.tensor_copy(xT[0], xT32[0].rearrange("p t b -> p (t b)"))
        nc.scalar.copy(xT[1], xT32[1].rearrange("p t b -> p (t b)"))

        # u~ = WiT.T @ x_t + WiTn.T @ x_{t-1}, copied psum -> u_sb per block;
        # copies round-robin DVE/Act/Pool so no single engine serializes.
        ntb = tval * BL
        nb_sz = 512
        nblocks = (ntb + nb_sz - 1) // nb_sz
        cp = 0
        for nb in range(nblocks):
            cnt = min(nb_sz, ntb - nb * nb_sz)
            tspan = cnt // BL
            t0 = nb * nb_sz // BL
            n0 = nb * nb_sz
            for m in range(MC):
                psu = pps.tile([128, nb_sz], F32, tag="psu")
                for ck in range(CIC):
                    nc.tensor.matmul(
                        psu[:, :cnt],
                        wiT[ck][:, 128 * m : 128 * (m + 1)],
                        xT[ck][:, n0 : n0 + cnt],
                        start=(ck == 0),
                        stop=False,
                    )
                for ck in range(CIC):
                    if nb == 0:
                        # t=0 has no t-1 term; shift the rest by one step
                        nc.tensor.matmul(
                            psu[:, BL:cnt],
                            wiTn[ck][:, 128 * m : 128 * (m + 1)],
                            xT[ck][:, : cnt - BL],
                            start=False,
                            stop=(ck == CIC - 1),
                        )
                    else:
                        nc.tensor.matmul(
                            psu[:, :cnt],
                            wiTn[ck][:, 128 * m : 128 * (m + 1)],
                            xT[ck][:, n0 - BL : n0 - BL + cnt],
                            start=False,
                            stop=(ck == CIC - 1),
                        )
                dst = u_sb[:, t0 : t0 + tspan, m, :]
                src = psu[:, :cnt].rearrange("p (t b) -> p t b", b=BL)
                # GPSIMD cannot read PSUM on HW; alternate DVE/Act only
                if cp % 2 == 0:
                    nc.vector.tensor_copy(dst, src)
                else:
                    nc.scalar.copy(dst, src)
                cp += 1


def esn_kernel(ctx, tc, x, wi, w, out, tval, dynamic):
    nc = tc.nc
    consts = ctx.enter_context(tc.tile_pool(name="consts", bufs=1))

    # identity (for u~ injection) and 0.5*identity (for the a-feedback)
    id16 = consts.tile([128, 128], F16, tag="id16", name="id16")
    make_identity(nc, id16)
    ih16 = consts.tile([128, 128], F16, tag="ih16", name="ih16")
    nc.vector.tensor_scalar_mul(ih16, id16, 0.5)

    # u~ precompute first: its wi/x DMAs are small and unblock the PE-side
    # U work; the W DMAs (issued right after the x DMAs, below) then overlap
    # with the U compute on the DMA engines.
    # 2 pad steps so the pair-ahead ut2 prefetch can read past the end.
    u_sb = consts.tile([128, tval + 2, MC, BL], F16, tag="usb")
    nc.vector.memset(u_sb[:, tval : tval + 2, :, :], 0.0)

    # W staging lives in consts (not the precompute pool) so its DMA has no
    # write-after-read wait on precompute tiles; split in two so converts
    # start at the halfway point.
    w32h = [
        consts.tile([128, KC // 2, R], F32, tag=f"w32{j}", name=f"w32{j}")
        for j in range(2)
    ]

    def issue_w_dmas():
        for j in range(2):
            nc.scalar.dma_start(
                out=w32h[j],
                in_=w[4 * 128 * j : 4 * 128 * (j + 1), :].rearrange(
                    "(k p) r -> p k r", p=128
                ),
            )

    _precompute_u(ctx, tc, x, wi, u_sb, tval, after_x_dmas=issue_w_dmas)

    # W/2, fp16, resident: 8 tiles [128, 1024]; converts alternate DVE/Act
    w_tiles = []
    for k in range(KC):
        wt = consts.tile([128, R], F16, tag=f"w{k}", name=f"w{k}")
        src = w32h[k // 4][:, k % 4, :]
        if k % 2 == 0:
            nc.vector.tensor_scalar_mul(wt, src, 0.5)
        else:
            nc.scalar.activation(wt, src, AF.Copy, scale=0.5)
        w_tiles.append(wt)

    ppool = ctx.enter_context(tc.tile_pool(name="ps", bufs=2, space="PSUM"))
    upool = ctx.enter_context(tc.tile_pool(name="ut", bufs=2))

    # Row-group pipelining: the ROWS=16 (b,c) rows are independent
    # recurrences. Split them into groups; each group has its own psum /
    # tanh / state tiles (separate tiles because Tile tracks deps per
    # tile), so group X's tanh chain overlaps the other groups' matmuls.
    rbounds = [
        int(v) for v in os.environ.get("ESN_RB", "0,8,16").split(",")
    ]
    NG = len(rbounds) - 1
    gsz = [rbounds[i + 1] - rbounds[i] for i in range(NG)]

    # per-parity, per-group state tiles: g (tanh out), a16 (psum mirror);
    # free layout [m, row-in-group]
    g2 = [
        [
            consts.tile([128, MC * gsz[gi]], F16, tag=f"g{j}{gi}", name=f"g{j}{gi}")
            for gi in range(NG)
        ]
        for j in range(2)
    ]
    a2 = [
        [
            consts.tile([128, MC * gsz[gi]], F16, tag=f"a{j}{gi}", name=f"a{j}{gi}")
            for gi in range(NG)
        ]
        for j in range(2)
    ]
    for gi in range(NG):
        nc.vector.memset(g2[1][gi], 0.0)
        nc.vector.memset(a2[1][gi], 0.0)

    # output slabs (pair index mod NSLAB); layout [p, (m, q, row)].
    # >2 slabs so the DMA's slow semaphore (+900ns) never gates the loop.
    NSLAB = 4
    trb = [
        consts.tile([128, MC * 2 * ROWS], F16, tag=f"trb{j}", name=f"trb{j}")
        for j in range(NSLAB)
    ]
    nc.vector.memset(trb[NSLAB - 1], 0.0)

    def step(t_par, q, ut2, trb_cur, trb_prev):
        # t parity: reads g2/a2[1-t_par], writes [t_par]
        gp, gc = g2[1 - t_par], g2[t_par]
        ap_, ac = a2[1 - t_par], a2[t_par]
        psg = [
            ppool.tile([128, MC * gsz[gi]], F32, tag=f"ps{gi}", name=f"ps{gi}")
            for gi in range(NG)
        ]

        def umm(gi, m):
            # u~ injection; may need several matmuls per group if the row
            # range crosses batch boundaries (c broadcast is per-batch)
            r0, r1 = rbounds[gi], rbounds[gi + 1]
            w = gsz[gi]
            first = m == 0
            r = r0
            while r < r1:
                b = r // C
                c0 = r % C
                cn = min(C - c0, r1 - r)
                uv = ut2[:, q : q + 1, m : m + 1, b : b + 1]
                uvb = bass.AP(uv.tensor, uv.offset, list(uv.ap) + [[0, cn]])
                nc.tensor.matmul(
                    psg[gi][:, w * m + (r - r0) : w * m + (r - r0) + cn],
                    id16,
                    uvb,
                    start=first,
                    stop=False,
                )
                first = False
                r += cn

        def amm(gi, m):
            w = gsz[gi]
            nc.tensor.matmul(
                psg[gi][:, w * m : w * (m + 1)],
                ih16,
                ap_[gi][:, w * m : w * (m + 1)],
                start=False,
                stop=False,
            )

        def gmm(gi, k, m):
            w = gsz[gi]
            nc.tensor.matmul(
                psg[gi][:, w * m : w * (m + 1)],
                w_tiles[k][:, 128 * m : 128 * (m + 1)],
                gp[gi][:, w * k : w * (k + 1)],
                start=False,
                stop=(k == KC - 1 and m == MC - 1),
            )

        # per group: u-mms first (dep ready at pair start), g-matmuls next
        # (need this group's previous tanh), and the a-feedback mm tucked
        # just before the last k chunk (its DVE mirror lands mid-step);
        # groups run in order so group gi's tanh chain overlaps the other
        # groups' matmuls.
        for gi in range(NG):
            for m in range(MC):
                umm(gi, m)
        for gi in range(NG):
            for k in range(KC - 1):
                for m in range(MC):
                    gmm(gi, k, m)
            for m in range(MC):
                amm(gi, m)
            for m in range(MC):
                gmm(gi, KC - 1, m)
            # psum for group gi complete (GPSIMD cannot read PSUM on HW,
            # so the fp16 mirror for the a-feedback goes through DVE)
            nc.scalar.activation(gc[gi], psg[gi], AF.Tanh)
            nc.vector.tensor_copy(ac[gi], psg[gi])

        trv_c = trb_cur.rearrange("p (m q2 row) -> p m q2 row", q2=2, row=ROWS)
        if q == 0:
            sprev = trb_prev.rearrange(
                "p (m q2 row) -> p m q2 row", q2=2, row=ROWS
            )[:, :, 1, :]
        else:
            sprev = trv_c[:, :, 0, :]
        for gi in range(NG):
            r0, r1 = rbounds[gi], rbounds[gi + 1]
            # output blend: s_t = 0.5*s_{t-1} + g_t, straight into the slab
            # (walrus rejects TensorScalarPtr on Pool, so DVE)
            nc.vector.scalar_tensor_tensor(
                out=trv_c[:, :, q, r0:r1],
                in0=sprev[:, :, r0:r1],
                scalar=0.5,
                in1=gc[gi].rearrange("p (m row) -> p m row", row=gsz[gi]),
                op0=ALU.mult,
                op1=ALU.add,
            )

    # ut2 staging is prefetched one pair ahead (ping-pong tiles by pair
    # parity) so the u-matmuls at the head of each pair never wait on it
    ut2s = [
        consts.tile([128, 2, MC, BL], F16, tag=f"ut2{j}", name=f"ut2{j}")
        for j in range(2)
    ]
    nc.gpsimd.tensor_copy(ut2s[0], u_sb[:, 0:2, :, :])

    def pair_body(iv, pair_par):
        trb_cur, trb_prev = trb[pair_par], trb[(pair_par - 1) % NSLAB]
        ut2 = ut2s[pair_par % 2]
        ut2_next = ut2s[(pair_par + 1) % 2]
        if dynamic:
            nc.gpsimd.tensor_copy(
                ut2_next, u_sb[:, bass.ds(iv * 2 + 2, 2), :, :]
            )
        else:
            nc.gpsimd.tensor_copy(
                ut2_next, u_sb[:, 2 * iv + 2 : 2 * iv + 4, :, :]
            )
        step(0, 0, ut2, trb_cur, trb_prev)
        step(1, 1, ut2, trb_cur, trb_prev)
        if dynamic:
            dst = out[bass.ts(iv, 1), :, :]
        else:
            dst = out[iv : iv + 1, :, :]
        nc.sync.dma_start(out=dst, in_=trb_cur)

    npairs = tval // 2
    loop_mode = os.environ.get("ESN_LOOP", "stag")
    stag_n = int(os.environ.get("ESN_STAGN", "4"))
    if dynamic and loop_mode == "stag" and npairs % stag_n == 0:
        with tc.For_i(
            0,
            npairs // stag_n,
            1,
            staggered_reset=True,
            hint_engines=(mybir.EngineType.PE,),
        ) as li:
            for j in range(stag_n):
                pair_body(li * stag_n + j, j % NSLAB)
    elif dynamic:
        tc.For_i_unrolled_general(
            0,
            npairs,
            1,
            lambda iv0, u: [pair_body(iv0 + j, j % NSLAB) for j in range(u)],
            max_unroll=int(os.environ.get("ESN_UNROLL", "2")),
            hint_engines=(mybir.EngineType.PE,),
        )
    else:
        for iv in range(npairs):
            pair_body(iv, iv % NSLAB)


def build_nc(tval=T, dynamic=True):
    nc = bass.Bass("TRN2", target_bir_lowering=False, debug=False)
    x_t = nc.dram_tensor("x", [BL, C, tval, I], F32, kind="ExternalInput")
    wi_t = nc.dram_tensor("wi", [R, CI], F32, kind="ExternalInput")
    w_t = nc.dram_tensor("w", [R, R], F32, kind="ExternalInput")
    out_t = nc.dram_tensor(
        "out", [tval // 2, 128, 2 * MC * ROWS], F16, kind="ExternalOutput"
    )
    with tile.TileContext(nc) as tc, ExitStack() as ctx:
        esn_kernel(
            ctx, tc, x_t.ap(), wi_t.ap(), w_t.ap(), out_t.ap(), tval, dynamic
        )
    return nc


def unscramble(arr, tval):
    """[T/2, 128, (m q row)] fp16 device layout -> h = s/2 as [BL, C, T, R]."""
    a = np.asarray(arr).reshape(tval // 2, 128, MC, 2, ROWS)
    a = a.transpose(4, 0, 3, 2, 1)  # [row, pair, q, m, p]
    a = a.reshape(ROWS, tval, R).astype(np.float32) * 0.5
    return np.ascontiguousarray(a).reshape(BL, C, tval, R)


def kernel(x, input_weights, reservoir_weights):
    x = np.ascontiguousarray(np.asarray(x, dtype=np.float32))
    wi = np.ascontiguousarray(np.asarray(input_weights, dtype=np.float32))
    w = np.ascontiguousarray(np.asarray(reservoir_weights, dtype=np.float32))

    from concourse.bass_utils import run_bass_kernel_spmd

    nc = build_nc(T, dynamic=True)
    _split_sync_waits(nc)
    in_maps = [
        {"x": x[BL * c : BL * (c + 1)], "wi": wi, "w": w} for c in range(NCORES)
    ]
    res = run_bass_kernel_spmd(nc, in_maps, core_ids=list(range(NCORES)))
    outs = [unscramble(np.asarray(m["out"]), T) for m in res.results]
    return np.concatenate(outs, axis=0)


if __name__ == "__main__":
    import jax

    with jax.default_device(jax.devices("cpu")[0]):
        import reference

        inputs = reference.setup_inputs()
        expected = np.asarray(reference.reference(**inputs))
    actual = kernel(**{k: np.asarray(v) for k, v in inputs.items()})
    err = np.abs(actual - expected).max()
    rel = err / max(1e-30, np.abs(expected).max())
    print(f"absmax err {err:.3e}  rel {rel:.3e}")


# revision 3
# speedup vs baseline: 1.0270x; 1.0270x over previous
"""Echo State Network Bass kernel for Trainium2 (8 NeuronCores, SPMD) — v3.

Problem: x [B=32, C=4, T=512, I=64], input_weights Wi [R=1024, C*I=256],
reservoir_weights W [R=1024, R]. Output [B, C, T, R] f32.

    u_t = flatten(x[:,:,t,:]) @ Wi.T                     (broadcast over C)
    h_t = 0.5*tanh(u_t + h_{t-1} @ W) + 0.5*h_{t-1}      (per (b, c) row)

Sharding: data-parallel over batch, 4 batches/core -> ROWS = 16 independent
reservoir rows per core; time recurrence local.

v3 device algorithm ("pre-activation recurrence", all fp16 in the loop):
  Let a_t = u_t + h_{t-1}@W (the tanh argument), g_t = tanh(a_t),
  s_t = 2*h_t. Then:
      a_{t+1} = u~_{t+1} + g_t@(W/2) + 0.5*a_t,  u~_t = u_t - 0.5*u_{t-1}
      s_t     = 0.5*s_{t-1} + g_t                (output: h = s/2, on host)
  Per step the PSUM accumulation is: identity-matmuls of the staged u~
  slice, W/2-matmuls of g_{t-1} (transposed layout, r on partitions), and
  0.5*identity-matmuls of a16_{t-1} (an fp16 SBUF mirror of the previous
  PSUM, copied off by the *GPSIMD* engine, whose short latency keeps the
  a-feedback off the long Act chain). Act only does the two half-width
  tanhs; DVE only does one fused scalar_tensor_tensor blend per step
  (written straight into the fp16 DMA slab) plus one staging copy per pair.
  The host undoes the slab layout, halves, and converts to fp32.
"""

import os
import sys

import numpy as np

sys.path.insert(0, "/opt/trn_rl_repo")

from contextlib import ExitStack

import concourse.bass as bass
import concourse.tile as tile
from concourse import mybir
from concourse.masks import make_identity

F32 = mybir.dt.float32
F16 = mybir.dt.float16
AF = mybir.ActivationFunctionType
ALU = mybir.AluOpType


def _patched_drain_and_barrier(self, tick_clock, wait_clock):
    # The stock kernel-tail drain carries one sync-wait per touched semaphore;
    # this walrus build caps sync waits per TPB_CTRL instruction, so chunk the
    # waits across several sequential drains on the sync engine.
    from concourse.vector_clock import ScopedClock

    nc = self.nc
    carrier = nc.sync.drain()
    wait_clock.add_sem_waits(
        carrier.ins, ScopedClock({None: tick_clock.global_clock})
    )
    si = carrier.ins.sync_info
    waits = list(si.on_wait) if si is not None else []
    if len(waits) > 1:
        carrier.ins.sync_info.on_wait = waits[:1]
        for w in waits[1:]:
            d2 = nc.sync.drain()
            d2.ins.sync_info = mybir.SyncInfo(on_wait=[w], on_update=[])
    nc.all_engine_barrier()
    popped = nc._tile_sem_poison_stack.pop()
    assert popped is self._sem_poison
    nc.clear_and_free_semaphores(list(self.sems.allocated().values()))
    nc.all_engine_barrier()


tile.TileContext._drain_and_barrier = _patched_drain_and_barrier

_MAX_SYNC_WAITS = 1


def _split_sync_waits(nc):
    """This walrus build rejects instructions carrying more than a couple of
    sync waits. Move excess waits onto same-engine NoOp carriers inserted
    immediately before the instruction (sem thresholds are absolute, so
    waiting earlier in the same engine stream is equivalent)."""
    import copy

    scratch = bass.Bass("TRN2", target_bir_lowering=False, debug=False)
    with scratch.Block() as blk:

        @blk.sync
        def _(sync):
            sync.nop(hint="waitcarrier")

    template = None
    for bb in scratch.m.functions[0].blocks:
        for i in bb.instructions:
            if i.opcode == "NoOp":
                template = i
    assert template is not None

    n_added = 0
    for f in nc.m.functions:
        for bb in f.blocks:
            out = []
            for inst in bb.instructions:
                si = inst.sync_info
                waits = list(si.on_wait) if si is not None else []
                if len(waits) > _MAX_SYNC_WAITS:
                    extra = waits[: -_MAX_SYNC_WAITS]
                    for w in extra:
                        nop = copy.copy(template)
                        n_added += 1
                        nop.name = f"I-wsplit{n_added}"
                        nop.engine = inst.engine
                        nop.sync_info = mybir.SyncInfo(on_wait=[w], on_update=[])
                        out.append(nop)
                    inst.sync_info.on_wait = waits[-_MAX_SYNC_WAITS:]
                out.append(inst)
            if n_added:
                bb.instructions[:] = out
    return n_added


B, C, T, I, R = 32, 4, 512, 64, 1024
NCORES = 8
BL = B // NCORES          # 4 local batches per core
ROWS = BL * C             # 16 rows; row = b*C + c
KC = R // 128             # 8 contraction chunks
MC = R // 128             # 8 output blocks
CI = C * I                # 256
CIC = CI // 128           # 2 ci chunks


def _cbroad(ap4):
    """Append a stride-0 C dim to a [...,BL] AP -> rows (b, c) broadcast."""
    return bass.AP(ap4.tensor, ap4.offset, list(ap4.ap) + [[0, C]])


def _precompute_u(ctx, tc, x, wi, u_sb, tval, after_x_dmas=None):
    """u~ into SBUF fp16: u_sb[p, t, m, b] = u_t - 0.5*u_{t-1} at r=128m+p.

    The -0.5*u_{t-1} term is folded into the PE accumulation (a second set
    of matmuls against -0.5*WiT with the x operand shifted one step), so the
    psum holds u~ directly and one copy per block finishes the job."""
    nc = tc.nc
    with (
        tc.tile_pool(name="pre", bufs=1) as pre,
        tc.tile_pool(name="pps", bufs=4, space="PSUM") as pps,
    ):
        ident = pre.tile([128, 128], F32, tag="ident")
        make_identity(nc, ident)

        # Wi natural [r-block, ci] -> PE-transpose -> WiT fp16 [ci, r].
        # One merged DMA: per-instruction SP issue cost dominates small DMAs.
        win_all = pre.tile([128, MC, CI], F32, tag="win", name="win_all")
        nc.sync.dma_start(
            out=win_all, in_=wi.rearrange("(m p) c -> p m c", p=128)
        )
        win = [win_all[:, m, :] for m in range(MC)]
        wiT = [
            pre.tile([128, R], F16, tag=f"wit{c}", name=f"wit{c}")
            for c in range(CIC)
        ]
        for m in range(MC):
            for ck in range(CIC):
                pt = pps.tile([128, 128], F32, tag="pt")
                nc.tensor.transpose(pt, win[m][:, 128 * ck : 128 * (ck + 1)], ident)
                if ck == 0:
                    nc.vector.tensor_copy(wiT[ck][:, 128 * m : 128 * (m + 1)], pt)
                else:
                    nc.scalar.copy(wiT[ck][:, 128 * m : 128 * (m + 1)], pt)

        # xT fp16 [(c2 i) part, (t, b) free] per ci chunk; one DMA per
        # (chunk, channel-pair) instead of per batch
        xT32 = [
            pre.tile([128, tval, BL], F32, tag=f"xt{c}", name=f"xt{c}")
            for c in range(CIC)
        ]
        qs = [nc.sync, nc.scalar]
        di = 0
        for ck in range(CIC):
            for cv in range(2):
                for b in range(BL):
                    src = x[b, 2 * ck + cv, :, :].rearrange("t i -> i t")
                    qs[di % 2].dma_start(
                        out=xT32[ck][I * cv : I * (cv + 1), :, b], in_=src
                    )
                    di += 1
        if after_x_dmas is not None:
            after_x_dmas()
        xT = [
            pre.tile([128, tval * BL], F16, tag=f"xh{c}", name=f"xh{c}")
            for c in range(CIC)
        ]
        # converts on Pool (idle at startup; all-SBUF so HW-legal)
        nc.gpsimd.tensor_copy(xT[0], xT32[0].rearrange("p t b -> p (t b)"))
        nc.gpsimd.tensor_copy(xT[1], xT32[1].rearrange("p t b -> p (t b)"))

        # raw u = WiT.T @ x, copied psum -> u_sb per block (DVE/Act
        # alternating). No u~ transform: the loop's psum mirror subtracts
        # u_t on the fly (scalar_tensor_tensor), so raw u is what we want.
        ntb = tval * BL
        nb_sz = 512
        nblocks = (ntb + nb_sz - 1) // nb_sz
        cp = 0
        for nb in range(nblocks):
            cnt = min(nb_sz, ntb - nb * nb_sz)
            tspan = cnt // BL
            t0 = nb * nb_sz // BL
            n0 = nb * nb_sz
            for m in range(MC):
                psu = pps.tile([128, nb_sz], F32, tag="psu")
                for ck in range(CIC):
                    nc.tensor.matmul(
                        psu[:, :cnt],
                        wiT[ck][:, 128 * m : 128 * (m + 1)],
                        xT[ck][:, n0 : n0 + cnt],
                        start=(ck == 0),
                        stop=(ck == CIC - 1),
                    )
                dst = u_sb[:, t0 : t0 + tspan, :, m, :]
                src = psu[:, :cnt].rearrange(
                    "p (t g bg) -> p t g bg",
                    g=u_sb.shape[2],
                    bg=u_sb.shape[4],
                )
                # GPSIMD cannot read PSUM on HW; alternate DVE/Act only
                if cp % 2 == 0:
                    nc.vector.tensor_copy(dst, src)
                else:
                    nc.scalar.copy(dst, src)
                cp += 1


def esn_kernel(ctx, tc, x, wi, w, out, tval, dynamic):
    nc = tc.nc
    consts = ctx.enter_context(tc.tile_pool(name="consts", bufs=1))

    # identity (for u~ injection) and 0.5*identity (for the a-feedback)
    id16 = consts.tile([128, 128], F16, tag="id16", name="id16")
    make_identity(nc, id16)
    ih16 = consts.tile([128, 128], F16, tag="ih16", name="ih16")
    nc.vector.tensor_scalar_mul(ih16, id16, 0.5)

    # raw-u precompute first: its wi/x DMAs are small and unblock the
    # PE-side U work; the W DMAs (issued right after the x DMAs, below)
    # then overlap with the U compute on the DMA engines.
    # Group-major u layout [t, group, m, b-in-group] keeps every consumer
    # AP within the HW's 3D limit. 2 tail pad slots let the pair-ahead ut2
    # prefetch read past the end.
    rbounds = [int(v) for v in os.environ.get("ESN_RB", "0,8,16").split(",")]
    NG = len(rbounds) - 1
    gsz = [rbounds[i + 1] - rbounds[i] for i in range(NG)]
    assert all(g == gsz[0] and g % C == 0 for g in gsz), (
        "row groups must be uniform whole-batch blocks"
    )
    BLG = gsz[0] // C
    u_sb = consts.tile([128, tval + 2, NG, MC, BLG], F16, tag="usb")
    nc.vector.memset(u_sb[:, tval : tval + 2, :, :, :], 0.0)

    # W staging lives in consts (not the precompute pool) so its DMA has no
    # write-after-read wait on precompute tiles; split in two so converts
    # start at the halfway point.
    w32h = [
        consts.tile([128, KC // 2, R], F32, tag=f"w32{j}", name=f"w32{j}")
        for j in range(2)
    ]

    def issue_w_dma(j):
        nc.scalar.dma_start(
            out=w32h[j],
            in_=w[4 * 128 * j : 4 * 128 * (j + 1), :].rearrange(
                "(k p) r -> p k r", p=128
            ),
        )

    # first W half before the x DMAs, second after: keeps the DMA device
    # busy end-to-end without starving either the U compute or the W path
    issue_w_dma(0)
    _precompute_u(
        ctx, tc, x, wi, u_sb, tval, after_x_dmas=lambda: issue_w_dma(1)
    )

    # W/2, fp16, resident: 8 tiles [128, 1024]; converts on Pool (idle at
    # startup while DVE/Act drain the u copies)
    w_tiles = []
    for k in range(KC):
        wt = consts.tile([128, R], F16, tag=f"w{k}", name=f"w{k}")
        src = w32h[k // 4][:, k % 4, :]
        nc.gpsimd.tensor_scalar_mul(wt, src, 0.5)
        w_tiles.append(wt)

    ppool = ctx.enter_context(tc.tile_pool(name="ps", bufs=2, space="PSUM"))
    upool = ctx.enter_context(tc.tile_pool(name="ut", bufs=2))

    # Row-group pipelining: the ROWS=16 (b,c) rows are independent
    # recurrences. Split them into groups; each group has its own psum /
    # tanh / state tiles (separate tiles because Tile tracks deps per
    # tile), so group X's tanh chain overlaps the other groups' matmuls.
    # per-parity, per-group state tiles: g (tanh out), a16 (psum mirror);
    # free layout [m, row-in-group]
    g2 = [
        [
            consts.tile([128, MC * gsz[gi]], F16, tag=f"g{j}{gi}", name=f"g{j}{gi}")
            for gi in range(NG)
        ]
        for j in range(2)
    ]
    a2 = [
        [
            consts.tile([128, MC * gsz[gi]], F16, tag=f"a{j}{gi}", name=f"a{j}{gi}")
            for gi in range(NG)
        ]
        for j in range(2)
    ]
    for gi in range(NG):
        nc.vector.memset(g2[1][gi], 0.0)
        nc.vector.memset(a2[1][gi], 0.0)

    # output slabs (pair index mod NSLAB); layout [p, (m, q, row)].
    # >2 slabs so the DMA's slow semaphore (+900ns) never gates the loop.
    NSLAB = 4
    trb = [
        consts.tile([128, MC * 2 * ROWS], F16, tag=f"trb{j}", name=f"trb{j}")
        for j in range(NSLAB)
    ]
    nc.vector.memset(trb[NSLAB - 1], 0.0)

    def step(t_par, q, ut2, trb_cur, trb_prev):
        # t parity: reads g2/a2[1-t_par], writes [t_par]
        gp, gc = g2[1 - t_par], g2[t_par]
        ap_, ac = a2[1 - t_par], a2[t_par]
        psg = [
            ppool.tile([128, MC * gsz[gi]], F32, tag=f"ps{gi}", name=f"ps{gi}")
            for gi in range(NG)
        ]

        def umm(gi, m):
            # raw-u injection; rows (bg, c) with the c broadcast stride-0
            w = gsz[gi]
            uv = ut2[:, q, gi, m, :]
            uvb = bass.AP(uv.tensor, uv.offset, list(uv.ap) + [[0, C]])
            nc.tensor.matmul(
                psg[gi][:, w * m : w * (m + 1)],
                id16,
                uvb,
                start=(m == 0),
                stop=False,
            )

        def amm(gi, m):
            w = gsz[gi]
            nc.tensor.matmul(
                psg[gi][:, w * m : w * (m + 1)],
                ih16,
                ap_[gi][:, w * m : w * (m + 1)],
                start=False,
                stop=False,
            )

        def gmm(gi, k, m):
            w = gsz[gi]
            nc.tensor.matmul(
                psg[gi][:, w * m : w * (m + 1)],
                w_tiles[k][:, 128 * m : 128 * (m + 1)],
                gp[gi][:, w * k : w * (k + 1)],
                start=False,
                stop=(k == KC - 1 and m == MC - 1),
            )

        # per group: u-mms first (dep ready at pair start), g-matmuls next
        # (need this group's previous tanh), and the a-feedback mm tucked
        # just before the last k chunk (its DVE mirror lands mid-step);
        # groups run in order so group gi's tanh chain overlaps the other
        # groups' matmuls.
        amm_late = os.environ.get("ESN_AMM", "late") == "late"
        for gi in range(NG):
            for m in range(MC):
                umm(gi, m)
            if not amm_late:
                for m in range(MC):
                    amm(gi, m)
        for gi in range(NG):
            for k in range(KC - 1):
                for m in range(MC):
                    gmm(gi, k, m)
            if amm_late:
                for m in range(MC):
                    amm(gi, m)
            for m in range(MC):
                gmm(gi, KC - 1, m)
            # psum for group gi complete. The fp16 mirror for the
            # a-feedback (DVE; GPSIMD cannot read PSUM on HW) subtracts
            # u_t as it copies: a16 = psum - u_t, so u_sb stays raw u.
            nc.scalar.activation(gc[gi], psg[gi], AF.Tanh)
            # ut2 is group-major [q, gi, m, bg], so (m, bg) is one packed
            # dim and the stt stays 3D (HW limit) with the c-broadcast
            uq = ut2[:, q, gi, :, :]
            uvb = bass.AP(
                uq.tensor,
                uq.offset,
                [uq.ap[0], [1, MC * (gsz[gi] // C)], [0, C]],
            )
            nc.vector.scalar_tensor_tensor(
                out=ac[gi].rearrange("p (mb c) -> p mb c", c=C),
                in0=uvb,
                scalar=-1.0,
                in1=psg[gi].rearrange("p (mb c) -> p mb c", c=C),
                op0=ALU.mult,
                op1=ALU.add,
            )

        trv_c = trb_cur.rearrange("p (m q2 row) -> p m q2 row", q2=2, row=ROWS)
        if q == 0:
            sprev = trb_prev.rearrange(
                "p (m q2 row) -> p m q2 row", q2=2, row=ROWS
            )[:, :, 1, :]
        else:
            sprev = trv_c[:, :, 0, :]
        blend_eng = os.environ.get("ESN_BLEND", "pool")
        for gi in range(NG):
            r0, r1 = rbounds[gi], rbounds[gi + 1]
            # output blend: s_t = 0.5*s_{t-1} + g_t, straight into the slab
            gcv = gc[gi].rearrange("p (m row) -> p m row", row=gsz[gi])
            if blend_eng == "pool":
                # walrus rejects the fused scalar_tensor_tensor on Pool but
                # allows tensor_scalar/tensor_tensor: halve into the slab
                # slot, then add g in place (keeps DVE free for mirrors)
                nc.gpsimd.tensor_scalar_mul(
                    trv_c[:, :, q, r0:r1], sprev[:, :, r0:r1], 0.5
                )
                nc.gpsimd.tensor_tensor(
                    trv_c[:, :, q, r0:r1],
                    trv_c[:, :, q, r0:r1],
                    gcv,
                    ALU.add,
                )
            else:
                nc.vector.scalar_tensor_tensor(
                    out=trv_c[:, :, q, r0:r1],
                    in0=sprev[:, :, r0:r1],
                    scalar=0.5,
                    in1=gcv,
                    op0=ALU.mult,
                    op1=ALU.add,
                )

    # ut2 staging is prefetched one pair ahead (ping-pong tiles by pair
    # parity) so the u-matmuls at the head of each pair never wait on it.
    # The prefetch also applies the u~ transform: ut2 = u_t - 0.5*u_{t-1}
    # (two Pool ops; u_sb index t+1 holds u_t).
    ut2s = [
        consts.tile(
            [128, 2, NG, MC, BLG], F16, tag=f"ut2{j}", name=f"ut2{j}"
        )
        for j in range(2)
    ]

    def fetch_ut2(dst, iv):
        if isinstance(iv, int):
            src = u_sb[:, 2 * iv : 2 * iv + 2, :, :, :]
        else:
            src = u_sb[:, bass.ds(iv * 2, 2), :, :, :]
        nc.gpsimd.tensor_copy(dst, src)

    fetch_ut2(ut2s[0], 0)

    def pair_body(iv, pair_par):
        trb_cur, trb_prev = trb[pair_par], trb[(pair_par - 1) % NSLAB]
        ut2 = ut2s[pair_par % 2]
        ut2_next = ut2s[(pair_par + 1) % 2]
        fetch_ut2(ut2_next, iv + 1)
        step(0, 0, ut2, trb_cur, trb_prev)
        step(1, 1, ut2, trb_cur, trb_prev)
        if dynamic:
            dst = out[bass.ts(iv, 1), :, :]
        else:
            dst = out[iv : iv + 1, :, :]
        nc.sync.dma_start(out=dst, in_=trb_cur)

    npairs = tval // 2
    loop_mode = os.environ.get("ESN_LOOP", "stag")
    stag_n = int(os.environ.get("ESN_STAGN", "4"))
    if dynamic and loop_mode == "stag" and npairs % stag_n == 0:
        with tc.For_i(
            0,
            npairs // stag_n,
            1,
            staggered_reset=True,
            hint_engines=(mybir.EngineType.PE,),
        ) as li:
            for j in range(stag_n):
                pair_body(li * stag_n + j, j % NSLAB)
    elif dynamic:
        tc.For_i_unrolled_general(
            0,
            npairs,
            1,
            lambda iv0, u: [pair_body(iv0 + j, j % NSLAB) for j in range(u)],
            max_unroll=int(os.environ.get("ESN_UNROLL", "2")),
            hint_engines=(mybir.EngineType.PE,),
        )
    else:
        for iv in range(npairs):
            pair_body(iv, iv % NSLAB)


def build_nc(tval=T, dynamic=True):
    nc = bass.Bass("TRN2", target_bir_lowering=False, debug=False)
    x_t = nc.dram_tensor("x", [BL, C, tval, I], F32, kind="ExternalInput")
    wi_t = nc.dram_tensor("wi", [R, CI], F32, kind="ExternalInput")
    w_t = nc.dram_tensor("w", [R, R], F32, kind="ExternalInput")
    out_t = nc.dram_tensor(
        "out", [tval // 2, 128, 2 * MC * ROWS], F16, kind="ExternalOutput"
    )
    with tile.TileContext(nc) as tc, ExitStack() as ctx:
        esn_kernel(
            ctx, tc, x_t.ap(), wi_t.ap(), w_t.ap(), out_t.ap(), tval, dynamic
        )
    return nc


def unscramble(arr, tval):
    """[T/2, 128, (m q row)] fp16 device layout -> h = s/2 as [BL, C, T, R]."""
    a = np.asarray(arr).reshape(tval // 2, 128, MC, 2, ROWS)
    a = a.transpose(4, 0, 3, 2, 1)  # [row, pair, q, m, p]
    a = a.reshape(ROWS, tval, R).astype(np.float32) * 0.5
    return np.ascontiguousarray(a).reshape(BL, C, tval, R)


def kernel(x, input_weights, reservoir_weights):
    x = np.ascontiguousarray(np.asarray(x, dtype=np.float32))
    wi = np.ascontiguousarray(np.asarray(input_weights, dtype=np.float32))
    w = np.ascontiguousarray(np.asarray(reservoir_weights, dtype=np.float32))

    from concourse.bass_utils import run_bass_kernel_spmd

    nc = build_nc(T, dynamic=True)
    _split_sync_waits(nc)
    in_maps = [
        {"x": x[BL * c : BL * (c + 1)], "wi": wi, "w": w} for c in range(NCORES)
    ]
    res = run_bass_kernel_spmd(nc, in_maps, core_ids=list(range(NCORES)))
    outs = [unscramble(np.asarray(m["out"]), T) for m in res.results]
    return np.concatenate(outs, axis=0)


if __name__ == "__main__":
    import jax

    with jax.default_device(jax.devices("cpu")[0]):
        import reference

        inputs = reference.setup_inputs()
        expected = np.asarray(reference.reference(**inputs))
    actual = kernel(**{k: np.asarray(v) for k, v in inputs.items()})
    err = np.abs(actual - expected).max()
    rel = err / max(1e-30, np.abs(expected).max())
    print(f"absmax err {err:.3e}  rel {rel:.3e}")
